# revision 1
# baseline (speedup 1.0000x reference)
"""Trainium2 Bass kernel for column self-attention (nn_ColumnSelfAttention).

Reference computation (per column c, columns are independent attention
problems):
    q = (x @ Wq + bq) * head_dim**-0.5 ; k = x @ Wk + bk ; v = x @ Wv + bv
    scores[h,c,i,j] = sum_d q[i,c,h,d] k[j,c,h,d]
    scores = where(mask[j,c], scores, -1e4); p = softmax_j(scores)
    ctx[i,c,:] = concat_h(p @ v) ; out = ctx @ Wo + bo

Sharding: the 256 columns are split across 8 NeuronCores (32 each).
Per core, tokens are ordered (column-major, row-inner) so one 128-token
tile == one column.  All matmul inputs are fp16 (fp32 PSUM accumulation);
softmax runs in fp32 on the scalar/vector engines.

Layout trick: scores are computed transposed (S_T[j,i]) so the key mask
becomes a per-partition bias fused into the Exp activation, and the
softmax denominator comes for free from an appended ones-column on V
(out[:, 64] of each head block = row sums).  The only transpose needed
is x (done by DMA-transpose on load) and the normalized context (PE
transpose via identity matmul) before the output projection.
"""

import os
import numpy as np

import concourse.bacc as bacc
import concourse.tile as tile
import concourse.mybir as mybir
from concourse import bass
from concourse.bass_utils import run_bass_kernel_spmd

R, C, E, H, D = 128, 256, 768, 12, 64
NCORES = 8
CLOC = C // NCORES            # 32 columns per core
BLK = 4                       # columns per block
NBLK = CLOC // BLK
T = BLK * R                   # 512 tokens per block
NTOK = CLOC * R               # 4096 tokens per core
NCH = E // 128                # 6 chunks of the embedding dim
F16 = mybir.dt.float16
F32 = mybir.dt.float32
Act = mybir.ActivationFunctionType

LAST_RESULTS = None           # for test.py introspection


def build_program(with_bias: bool, nblocks: int = NBLK, stage: int = 8):
    nc = bacc.Bacc("TRN2", target_bir_lowering=False, debug=False)

    # x^T per core, host-pretransposed: x_d[ec, p, t] = x[t, ec*128+p]
    x_d = nc.dram_tensor("x", [NCH, 128, NTOK], F16, kind="ExternalInput")
    madd_d = nc.dram_tensor("madd", [R, CLOC], F32, kind="ExternalInput")
    ident_d = nc.dram_tensor("ident", [128, 128], F16, kind="ExternalInput")
    w_d = {
        n: nc.dram_tensor(n, [E, E], F16, kind="ExternalInput")
        for n in ("wq", "wk", "wv", "wo")
    }
    if with_bias:
        bqk_d = {
            n: nc.dram_tensor(n, [128, NCH], F32, kind="ExternalInput")
            for n in ("bq", "bk")
        }
        bvo_d = {
            n: nc.dram_tensor(n, [1, E], F16, kind="ExternalInput")
            for n in ("bv", "bo")
        }
    o_d = nc.dram_tensor("o", [nblocks * BLK, R, E], F32, kind="ExternalOutput")

    with tile.TileContext(nc) as tc:
        with (
            tc.tile_pool(name="const", bufs=1) as const,
            tc.tile_pool(name="blk", bufs=2) as blkp,
            tc.tile_pool(name="col", bufs=4) as colp,
            tc.tile_pool(name="psmm", bufs=2, space="PSUM") as psmm,
            tc.tile_pool(name="pss", bufs=3, space="PSUM") as pssp,
            tc.tile_pool(name="pscx", bufs=3, space="PSUM") as pscx,
        ):
            # block-0 x tiles first so the first projection matmul isn't
            # stuck behind 4.7MB of weight DMAs
            w_sb = {}
            for n in ("wq", "wk", "wv", "wo"):
                w_sb[n] = const.tile([128, NCH, E], F16, tag=n, name=f"w_{n}")
            madd_sb = const.tile([R, CLOC], F32, tag="madd")
            ident_sb = const.tile([128, 128], F16, tag="ident")
            xt0 = blkp.tile([128, NCH, T], F16, tag="xt", name="xt0")
            # NOTE: strided multi-dim DMAs on the HWDGE (sync/scalar)
            # queues cause a chip-wide ~18% slowdown (observed twice);
            # keep all strided loads on the gpsimd SWDGE queue.
            nc.gpsimd.dma_start(
                w_sb["wq"][:], w_d["wq"].ap().rearrange("(c p) e -> p c e", p=128)
            )

            for ec in range(NCH):
                eng = nc.sync if ec % 2 == 0 else nc.scalar
                eng.dma_start(out=xt0[:, ec, :], in_=x_d.ap()[ec, :, 0:T])
            nc.gpsimd.dma_start(
                w_sb["wk"][:], w_d["wk"].ap().rearrange("(c p) e -> p c e", p=128)
            )
            nc.gpsimd.dma_start(madd_sb[:], madd_d.ap())
            nc.gpsimd.dma_start(ident_sb[:], ident_d.ap())
            nc.gpsimd.dma_start(
                w_sb["wv"][:], w_d["wv"].ap().rearrange("(c p) e -> p c e", p=128)
            )
            nc.gpsimd.dma_start(
                w_sb["wo"][:], w_d["wo"].ap().rearrange("(c p) e -> p c e", p=128)
            )
            if with_bias:
                bqk_sb = {}
                for n in ("bq", "bk"):
                    bqk_sb[n] = const.tile([128, NCH], F32, tag=n, name=f"b_{n}")
                    nc.gpsimd.dma_start(bqk_sb[n][:], bqk_d[n].ap())
                bvo_sb = {}
                for n in ("bv", "bo"):
                    bvo_sb[n] = const.tile([1, E], F16, tag=n, name=f"b_{n}")
                    nc.gpsimd.dma_start(bvo_sb[n][:], bvo_d[n].ap())
                ones_sb = const.tile([1, 128], F16, tag="ones")
                nc.gpsimd.memset(ones_sb[:], 1.0)

            pending_wo = None
            for b in range(nblocks):
                # ---- x^T for this block: (e, tok), plain contiguous DMA ----
                if b == 0:
                    xt = xt0
                else:
                    xt = blkp.tile([128, NCH, T], F16, tag="xt")
                    for ec in range(NCH):
                        eng = nc.sync if ec % 2 == 0 else nc.scalar
                        eng.dma_start(
                            out=xt[:, ec, :],
                            in_=x_d.ap()[ec, :, b * T : (b + 1) * T],
                        )

                if stage < 2:
                    continue
                # ---- Q^T, K^T projections: (e_out, tok) ----
                qt = blkp.tile([128, NCH, T], F16, tag="qt")
                kt = blkp.tile([128, NCH, T], F16, tag="kt")
                for wname, bname, dst in (("wq", "bq", qt), ("wk", "bk", kt)):
                    for co in range(NCH):
                        ps = psmm.tile([128, T], F32, tag="mm")
                        for k in range(NCH):
                            nc.tensor.matmul(
                                ps[:],
                                w_sb[wname][:, k, co * 128 : (co + 1) * 128],
                                xt[:, k, :],
                                start=(k == 0),
                                stop=(k == NCH - 1),
                            )
                        if with_bias:
                            nc.scalar.activation(
                                dst[:, co, :], ps[:], Act.Identity,
                                bias=bqk_sb[bname][:, co : co + 1],
                            )
                        else:
                            nc.scalar.copy(dst[:, co, :], ps[:])

                if stage < 3:
                    continue
                # ---- per-head zero-padded Q^T (base-partition-64 matmuls
                # into shared PSUM banks crash HW; contract K=128 instead,
                # with the other head's rows zeroed on the Q side) ----
                qz = blkp.tile([128, H, T], F16, tag="qz")
                nc.gpsimd.memset(qz[:], 0.0)
                for h in range(H):
                    ch, off = divmod(h, 2)
                    off *= 64
                    nc.vector.tensor_copy(
                        qz[off : off + 64, h, :], qt[off : off + 64, ch, :]
                    )

                # ---- V projection, natural layout, interleaved with a ones
                # column per head: va[:, t, h*65:h*65+64] = V_h, [...,64] = 1 ----
                va = blkp.tile([128, BLK, H * 65], F16, tag="va")
                nc.gpsimd.memset(va[:], 1.0)
                for t in range(BLK):
                    for half in range(2):
                        psv = psmm.tile([128, 384], F32, tag="mm")
                        if with_bias:
                            nc.tensor.matmul(
                                psv[:], ones_sb[:],
                                bvo_sb["bv"][:, half * 384 : (half + 1) * 384],
                                start=True, stop=False,
                            )
                        for k in range(NCH):
                            nc.tensor.matmul(
                                psv[:],
                                xt[:, k, t * 128 : (t + 1) * 128],
                                w_sb["wv"][:, k, half * 384 : (half + 1) * 384],
                                start=(k == 0 and not with_bias),
                                stop=(k == NCH - 1),
                            )
                        dst = va[:, t, half * 390 : (half + 1) * 390]
                        dst = dst.rearrange("p (h x) -> p h x", x=65)[:, :, 0:64]
                        nc.vector.tensor_copy(
                            dst, psv[:].rearrange("p (h d) -> p h d", d=64)
                        )

                # ---- attention, software-pipelined across columns so the
                # PE always has independent work while the per-column
                # PV -> recip/normalize (DVE) -> transpose -> copy -> Wo
                # chain drains.  PE emission order per cycle t:
                #   PV(t), S(t+2), WO(t-1), TR(t)
                if stage < 4:
                    continue
                ets, pscs, ctxns, ctxnts = {}, {}, {}, {}

                def emit_scores(t):
                    cg = b * BLK + t
                    et = colp.tile([128, H * 128], F16, tag="et",
                                   name=f"et_{b}_{t}")
                    for g3 in range(3):
                        pss = pssp.tile([128, 512], F32, tag="s", name="pss")
                        for hh in range(4):
                            h = g3 * 4 + hh
                            nc.tensor.matmul(
                                pss[:, hh * 128 : (hh + 1) * 128],
                                kt[:, h // 2, t * 128 : (t + 1) * 128],
                                qz[:, h, t * 128 : (t + 1) * 128],
                                start=(hh == 0),
                                stop=(hh == 3),
                            )
                        nc.scalar.activation(
                            et[:, g3 * 512 : (g3 + 1) * 512], pss[:], Act.Exp,
                            bias=madd_sb[:, cg : cg + 1], scale=1.0,
                        )
                    ets[t] = et

                def emit_pv(t):
                    if stage < 5:
                        return
                    et = ets[t]
                    psc = []
                    for g2 in range(2):
                        pc = pscx.tile([128, 390], F32, tag="cx", name="pc")
                        for hh in range(6):
                            h = g2 * 6 + hh
                            nc.tensor.matmul(
                                pc[:, hh * 65 : (hh + 1) * 65],
                                et[:, h * 128 : (h + 1) * 128],
                                va[:, t, h * 65 : (h + 1) * 65],
                                start=(hh == 0),
                                stop=(hh == 5),
                            )
                        psc.append(pc)
                    pscs[t] = psc

                def emit_norm_tr(t):
                    if stage < 6:
                        return
                    psc = pscs[t]
                    recip = colp.tile([128, H], F32, tag="recip", name="recip")
                    ctxn = colp.tile([128, E], F16, tag="ctxn", name="ctxn")
                    for g2 in range(2):
                        grp = psc[g2].rearrange("p (h x) -> p h x", x=65)
                        nc.vector.reciprocal(
                            recip[:, g2 * 6 : (g2 + 1) * 6].unsqueeze(2),
                            grp[:, :, 64:65],
                        )
                        nc.vector.tensor_mul(
                            ctxn[:, g2 * 384 : (g2 + 1) * 384].rearrange(
                                "p (h d) -> p h d", d=64
                            ),
                            grp[:, :, 0:64],
                            recip[:, g2 * 6 : (g2 + 1) * 6]
                            .unsqueeze(2)
                            .broadcast_to((128, 6, 64)),
                        )
                    ctxns[t] = ctxn
                    if stage < 7:
                        return
                    pst = pscx.tile([128, NCH, 128], F16, tag="cx", name="pst")
                    for ec in range(NCH):
                        nc.tensor.transpose(
                            pst[:, ec, :],
                            ctxn[:, ec * 128 : (ec + 1) * 128],
                            ident_sb[:],
                        )
                    ctxnt = colp.tile([128, NCH, 128], F16, tag="ctxnt",
                                      name="ctxnt")
                    nc.vector.tensor_copy(ctxnt[:], pst[:])
                    ctxnts[t] = ctxnt

                def emit_wo(t, cg, store):
                    if stage < 8 or t not in store:
                        return
                    ctxnt = store.pop(t)
                    osb = colp.tile([128, E], F32, tag="osb", name="osb")
                    for half in range(2):
                        po = psmm.tile([128, 384], F32, tag="mm", name="po")
                        if with_bias:
                            nc.tensor.matmul(
                                po[:], ones_sb[:],
                                bvo_sb["bo"][:, half * 384 : (half + 1) * 384],
                                start=True, stop=False,
                            )
                        for k in range(NCH):
                            nc.tensor.matmul(
                                po[:],
                                ctxnt[:, k, :],
                                w_sb["wo"][:, k, half * 384 : (half + 1) * 384],
                                start=(k == 0 and not with_bias),
                                stop=(k == NCH - 1),
                            )
                        nc.scalar.copy(osb[:, half * 384 : (half + 1) * 384], po[:])
                    nc.gpsimd.dma_start(o_d.ap()[cg], osb[:])

                emit_scores(0)
                if BLK > 1:
                    emit_scores(1)
                for t in range(BLK):
                    emit_pv(t)
                    if t + 2 < BLK:
                        emit_scores(t + 2)
                    if t >= 1:
                        emit_wo(t - 1, b * BLK + t - 1, ctxnts)
                    elif pending_wo is not None:
                        pending_wo()           # last column of previous block
                        pending_wo = None
                    emit_norm_tr(t)
                import functools
                pending_wo = functools.partial(
                    emit_wo, BLK - 1, b * BLK + BLK - 1, ctxnts
                )

            if pending_wo is not None:
                pending_wo()
    nc.compile()
    return nc


_PROGRAMS = {}


def _get_program(with_bias: bool):
    if with_bias not in _PROGRAMS:
        _PROGRAMS[with_bias] = build_program(with_bias)
    return _PROGRAMS[with_bias]


def make_in_maps(x, self_attn_padding_mask, Wq, bq, Wk, bk, Wv, bv, Wo, bo,
                 with_bias):
    scaling = float(D) ** -0.5
    wq = np.ascontiguousarray((np.asarray(Wq, np.float32) * scaling).astype(np.float16))
    wk = np.ascontiguousarray(np.asarray(Wk, np.float32).astype(np.float16))
    wv = np.ascontiguousarray(np.asarray(Wv, np.float32).astype(np.float16))
    wo = np.ascontiguousarray(np.asarray(Wo, np.float32).astype(np.float16))
    mask = np.asarray(self_attn_padding_mask)[0]                   # (R, C)
    madd_full = np.where(mask, 0.0, -10000.0).astype(np.float32)   # (R, C)
    xf = np.asarray(x, np.float32)[:, :, 0, :]                     # (R, C, E)
    ident = np.eye(128, dtype=np.float16)
    in_maps = []
    for i in range(NCORES):
        cs = slice(i * CLOC, (i + 1) * CLOC)
        xs = (
            xf[:, cs]
            .transpose(1, 0, 2)                # (CLOC, R, E) tok-major
            .reshape(NTOK, NCH, 128)
            .transpose(1, 2, 0)                # (NCH, 128, NTOK) = x^T chunks
        )
        xs = np.ascontiguousarray(xs.astype(np.float16))
        m = {
            "x": xs,
            "madd": np.ascontiguousarray(madd_full[:, cs]),
            "wq": wq, "wk": wk, "wv": wv, "wo": wo,
            "ident": ident,
        }
        if with_bias:
            m["bq"] = np.ascontiguousarray(
                (np.asarray(bq, np.float32) * scaling).reshape(NCH, 128).T
            )
            m["bk"] = np.ascontiguousarray(
                np.asarray(bk, np.float32).reshape(NCH, 128).T
            )
            m["bv"] = np.asarray(bv, np.float32).astype(np.float16).reshape(1, E)
            m["bo"] = np.asarray(bo, np.float32).astype(np.float16).reshape(1, E)
        in_maps.append(m)
    return in_maps


def assemble_output(shards):
    out = np.empty((R, C, 1, E), np.float32)
    for i in range(NCORES):
        out[:, i * CLOC : (i + 1) * CLOC, 0, :] = shards[i].transpose(1, 0, 2)
    return out


def kernel(x, self_attn_padding_mask, Wq, bq, Wk, bk, Wv, bv, Wo, bo):
    global LAST_RESULTS
    with_bias = any(
        bool(np.any(np.asarray(b))) for b in (bq, bk, bv, bo)
    )
    nc = _get_program(with_bias)
    in_maps = make_in_maps(
        x, self_attn_padding_mask, Wq, bq, Wk, bk, Wv, bv, Wo, bo, with_bias
    )
    trace = os.environ.get("KERNEL_TRACE", "") not in ("", "0")
    res = run_bass_kernel_spmd(
        nc, in_maps, core_ids=list(range(NCORES)), trace=trace
    )
    LAST_RESULTS = res
    return assemble_output([res.results[i]["o"] for i in range(NCORES)])



# revision 5
# speedup vs baseline: 1.0721x; 1.0721x over previous
"""Trainium2 Bass kernel for column self-attention (nn_ColumnSelfAttention).

Reference computation (per column c, columns are independent attention
problems):
    q = (x @ Wq + bq) * head_dim**-0.5 ; k = x @ Wk + bk ; v = x @ Wv + bv
    scores[h,c,i,j] = sum_d q[i,c,h,d] k[j,c,h,d]
    scores = where(mask[j,c], scores, -1e4); p = softmax_j(scores)
    ctx[i,c,:] = concat_h(p @ v) ; out = ctx @ Wo + bo

Sharding: 256 columns split across 8 NeuronCores.  Columns are sorted by
unmasked-key count and dealt round-robin so all cores share one static
program: position p on every core gets capacity KCAP[p] = max count among
the 8 columns of that rank group.

Sparsity: masked keys contribute exactly 0 after softmax (exp(-1e4) == 0
in the reference), so K and V are only projected for the *unmasked* keys
of each column (packed on host, zero-padded to KCAP[p]).  That removes
the mask entirely: scores of padding rows are exp(0)=1 but their V rows
and denominator ("ones") entries are 0.  K^T is used directly as the
scores stationary; V is computed transposed (tokens on the free axis so
packing pays) then flipped per column with a PE-identity transpose.

All matmul inputs are fp16 (fp32 PSUM accumulation); softmax is exp on
the scalar engine + reciprocal/mul on DVE, with the denominator coming
free from an appended ones-column on V.
"""

import os
import numpy as np

import concourse.bacc as bacc
import concourse.tile as tile
import concourse.mybir as mybir
from concourse import bass
from concourse.bass_utils import run_bass_kernel_spmd

R, C, E, H, D = 128, 256, 768, 12, 64
NCORES = 8
CLOC = C // NCORES            # 32 columns per core
BLK = 4                       # columns per block
NBLK = CLOC // BLK
T = BLK * R                   # 512 q-tokens per block
NTOK = CLOC * R               # 4096 q-tokens per core
NCH = E // 128                # 6 chunks of the embedding dim
F16 = mybir.dt.float16
F32 = mybir.dt.float32
Act = mybir.ActivationFunctionType

LAST_RESULTS = None           # for test.py introspection


def build_program(with_bias: bool, kcap):
    """kcap: tuple of CLOC ints -- packed-key capacity per column position."""
    kcap = list(kcap)
    assert len(kcap) == CLOC and all(1 <= k <= 128 for k in kcap)
    off = [0]
    for k in kcap:
        off.append(off[-1] + k)
    TOT = off[-1]
    tb = [off[(b + 1) * BLK] - off[b * BLK] for b in range(NBLK)]  # packed toks/blk
    TBMAX = max(tb)

    nc = bacc.Bacc("TRN2", target_bir_lowering=False, debug=False)

    # x^T per core, host-pretransposed: x_d[ec, p, t] = x[t, ec*128+p]
    x_d = nc.dram_tensor("x", [NCH, 128, NTOK], F16, kind="ExternalInput")
    xkv_d = nc.dram_tensor("xkv", [NCH, 128, TOT], F16, kind="ExternalInput")
    vones_d = nc.dram_tensor("vones", [128, CLOC], F16, kind="ExternalInput")
    ident_d = nc.dram_tensor("ident", [128, 128], F16, kind="ExternalInput")
    # weights host-prearranged per 128-row chunk: w_d[n][k] = W[k*128:(k+1)*128, :]
    w_d = {
        n: nc.dram_tensor(n, [NCH, 128, E], F16, kind="ExternalInput")
        for n in ("wq", "wk", "wv", "wo")
    }
    if with_bias:
        bqkv_d = {
            n: nc.dram_tensor(n, [128, NCH], F32, kind="ExternalInput")
            for n in ("bq", "bk", "bv")
        }
        bo_d = nc.dram_tensor("bo", [1, E], F16, kind="ExternalInput")
    o_d = nc.dram_tensor("o", [CLOC, R, E], F32, kind="ExternalOutput")

    with tile.TileContext(nc) as tc:
        with (
            tc.tile_pool(name="const", bufs=1) as const,
            tc.tile_pool(name="blk", bufs=2) as blkp,
            tc.tile_pool(name="col", bufs=4) as colp,
            tc.tile_pool(name="psmm", bufs=2, space="PSUM") as psmm,
            tc.tile_pool(name="pss", bufs=2, space="PSUM") as pssp,
            tc.tile_pool(name="pscx", bufs=2, space="PSUM") as pscx,
            tc.tile_pool(name="pstr", bufs=2, space="PSUM") as pstr,
        ):
            # ---- constants / weights.  All DMAs here are contiguous 2D
            # (host prearranged), so the fast HWDGE queues are safe; the
            # chip-wide slowdown note only applies to strided multi-dim
            # descriptors.  Weight chunks are separate tiles so the first
            # projection matmul only waits for the chunks it reads.
            w_sb = {
                n: [const.tile([128, E], F16, tag=f"{n}{k}", name=f"w_{n}{k}")
                    for k in range(NCH)]
                for n in ("wq", "wk", "wv", "wo")
            }
            vones_sb = const.tile([128, CLOC], F16, tag="vones")
            ident_sb = const.tile([128, 128], F16, tag="ident")
            xt0 = blkp.tile([128, NCH, T], F16, tag="xt", name="xt0")
            xk0 = blkp.tile([128, NCH, TBMAX], F16, tag="xk", name="xk0")
            qs = (nc.gpsimd, nc.sync, nc.scalar)
            for ec in range(NCH):
                eng = nc.sync if ec % 2 == 0 else nc.scalar
                eng.dma_start(out=xt0[:, ec, :], in_=x_d.ap()[ec, :, 0:T])
            for k in range(NCH):
                qs[k % 3].dma_start(w_sb["wq"][k][:], w_d["wq"].ap()[k])
            for ec in range(NCH):  # first needed right after Q projection
                eng = nc.sync if ec % 2 == 0 else nc.scalar
                eng.dma_start(out=xk0[:, ec, 0:tb[0]], in_=xkv_d.ap()[ec, :, 0:tb[0]])
            for k in range(NCH):
                qs[k % 3].dma_start(w_sb["wk"][k][:], w_d["wk"].ap()[k])
            nc.gpsimd.dma_start(ident_sb[:], ident_d.ap())
            nc.gpsimd.dma_start(vones_sb[:], vones_d.ap())
            for k in range(NCH):
                qs[k % 3].dma_start(w_sb["wv"][k][:], w_d["wv"].ap()[k])
            for k in range(NCH):
                qs[k % 3].dma_start(w_sb["wo"][k][:], w_d["wo"].ap()[k])
            if with_bias:
                bqkv_sb = {}
                for n in ("bq", "bk", "bv"):
                    bqkv_sb[n] = const.tile([128, NCH], F32, tag=n, name=f"b_{n}")
                    nc.gpsimd.dma_start(bqkv_sb[n][:], bqkv_d[n].ap())
                bo_sb = const.tile([1, E], F16, tag="bo")
                nc.gpsimd.dma_start(bo_sb[:], bo_d.ap())
                ones_sb = const.tile([1, 128], F16, tag="ones")
                nc.gpsimd.memset(ones_sb[:], 1.0)

            pending_wo = None
            for b in range(NBLK):
                TB = tb[b]
                loff = [off[b * BLK + t] - off[b * BLK] for t in range(BLK)]

                if b == 0:
                    xt, xk = xt0, xk0
                else:
                    xt = blkp.tile([128, NCH, T], F16, tag="xt")
                    xk = blkp.tile([128, NCH, TBMAX], F16, tag="xk")
                    for ec in range(NCH):
                        eng = nc.sync if ec % 2 == 0 else nc.scalar
                        eng.dma_start(
                            out=xt[:, ec, :],
                            in_=x_d.ap()[ec, :, b * T : (b + 1) * T],
                        )
                    for ec in range(NCH):
                        eng = nc.sync if ec % 2 == 0 else nc.scalar
                        eng.dma_start(
                            out=xk[:, ec, 0:TB],
                            in_=xkv_d.ap()[ec, :, off[b * BLK] : off[b * BLK] + TB],
                        )

                # ---- Q^T over all tokens; K^T, V^T over packed keys ----
                qt = blkp.tile([128, NCH, T], F16, tag="qt")
                kt = blkp.tile([128, NCH, TBMAX], F16, tag="kt")
                vt = blkp.tile([128, NCH, TBMAX], F16, tag="vt")
                for wname, bname, src, n_, dst, ceng in (
                    ("wq", "bq", xt, T, qt, nc.scalar),
                    ("wk", "bk", xk, TB, kt, nc.scalar),
                    ("wv", "bv", xk, TB, vt, nc.vector),
                ):
                    for co in range(NCH):
                        ps = psmm.tile([128, T], F32, tag="mm", name="ps")
                        for k in range(NCH):
                            nc.tensor.matmul(
                                ps[:, 0:n_],
                                w_sb[wname][k][:, co * 128 : (co + 1) * 128],
                                src[:, k, 0:n_],
                                start=(k == 0),
                                stop=(k == NCH - 1),
                            )
                        if with_bias:
                            nc.scalar.activation(
                                dst[:, co, 0:n_], ps[:, 0:n_], Act.Identity,
                                bias=bqkv_sb[bname][:, co : co + 1],
                            )
                        elif ceng is nc.scalar:
                            nc.scalar.copy(dst[:, co, 0:n_], ps[:, 0:n_])
                        else:
                            nc.vector.tensor_copy(dst[:, co, 0:n_], ps[:, 0:n_])

                # ---- per-head zero-padded Q^T (base-partition-64 matmuls
                # into shared PSUM banks crash HW; contract K=128 instead,
                # with the other head's rows zeroed on the Q side) ----
                qz = blkp.tile([128, H, T], F16, tag="qz")
                if b < 2:
                    nc.gpsimd.memset(qz[:], 0.0)  # pool ring: zeros persist
                for h in range(H):
                    ch, o2 = divmod(h, 2)
                    o2 *= 64
                    nc.vector.tensor_copy(
                        qz[o2 : o2 + 64, h, :], qt[o2 : o2 + 64, ch, :]
                    )

                va = blkp.tile([128, BLK, H * 65], F16, tag="va")
                ets, pscs, ctxns, ctxnts = {}, {}, {}, {}

                def emit_vtrans(t):
                    # V natural per column: PE transpose of packed V^T,
                    # then DVE assembly into 65-stride head slots + ones.
                    p = b * BLK + t
                    kc = kcap[p]
                    lo = loff[t]
                    vtp = pstr.tile([128, NCH, 128], F16, tag="tr", name="vtp")
                    for ec in range(NCH):
                        nc.tensor.transpose(
                            vtp[0:kc, ec, :],
                            vt[:, ec, lo : lo + kc],
                            ident_sb[:],
                        )
                    for ec in range(NCH):
                        dst = va[0:kc, t, ec * 130 : (ec + 1) * 130]
                        dst = dst.rearrange("p (h x) -> p h x", x=65)[:, :, 0:64]
                        src = vtp[0:kc, ec, :].rearrange("p (h d) -> p h d", d=64)
                        if with_bias:
                            # zero padding rows (v=bv there otherwise)
                            nc.vector.tensor_mul(
                                dst, src,
                                vones_sb[0:kc, p : p + 1]
                                .unsqueeze(2)
                                .broadcast_to((kc, 2, 64)),
                            )
                        else:
                            nc.vector.tensor_copy(dst, src)
                    ones_dst = va[0:kc, t, :].rearrange(
                        "p (h x) -> p h x", x=65
                    )[:, :, 64:65]
                    nc.vector.tensor_copy(
                        ones_dst,
                        vones_sb[0:kc, p : p + 1].unsqueeze(2).broadcast_to(
                            (kc, H, 1)
                        ),
                    )

                def emit_scores(t):
                    p = b * BLK + t
                    kc = kcap[p]
                    lo = loff[t]
                    et = colp.tile([128, H * 128], F16, tag="et",
                                   name=f"et_{b}_{t}")
                    for g3 in range(3):
                        pss = pssp.tile([128, 512], F32, tag="s", name="pss")
                        for hh in range(4):
                            h = g3 * 4 + hh
                            nc.tensor.matmul(
                                pss[0:kc, hh * 128 : (hh + 1) * 128],
                                kt[:, h // 2, lo : lo + kc],
                                qz[:, h, t * 128 : (t + 1) * 128],
                                start=(hh == 0),
                                stop=(hh == 3),
                            )
                        nc.scalar.activation(
                            et[0:kc, g3 * 512 : (g3 + 1) * 512],
                            pss[0:kc, :], Act.Exp,
                        )
                    ets[t] = et

                def emit_pv(t):
                    p = b * BLK + t
                    kc = kcap[p]
                    et = ets[t]
                    psc = []
                    for g2 in range(2):
                        pc = pscx.tile([128, 390], F32, tag="cx", name="pc")
                        for hh in range(6):
                            h = g2 * 6 + hh
                            nc.tensor.matmul(
                                pc[:, hh * 65 : (hh + 1) * 65],
                                et[0:kc, h * 128 : (h + 1) * 128],
                                va[0:kc, t, h * 65 : (h + 1) * 65],
                                start=(hh == 0),
                                stop=(hh == 5),
                            )
                        psc.append(pc)
                    pscs[t] = psc

                def emit_norm_tr(t):
                    psc = pscs[t]
                    recip = colp.tile([128, H], F32, tag="recip", name="recip")
                    ctxn = colp.tile([128, E], F16, tag="ctxn", name="ctxn")
                    for g2 in range(2):
                        grp = psc[g2].rearrange("p (h x) -> p h x", x=65)
                        nc.vector.reciprocal(
                            recip[:, g2 * 6 : (g2 + 1) * 6].unsqueeze(2),
                            grp[:, :, 64:65],
                        )
                        nc.vector.tensor_mul(
                            ctxn[:, g2 * 384 : (g2 + 1) * 384].rearrange(
                                "p (h d) -> p h d", d=64
                            ),
                            grp[:, :, 0:64],
                            recip[:, g2 * 6 : (g2 + 1) * 6]
                            .unsqueeze(2)
                            .broadcast_to((128, 6, 64)),
                        )
                    ctxns[t] = ctxn
                    pst = pstr.tile([128, NCH, 128], F16, tag="tr", name="pst")
                    for ec in range(NCH):
                        nc.tensor.transpose(
                            pst[:, ec, :],
                            ctxn[:, ec * 128 : (ec + 1) * 128],
                            ident_sb[:],
                        )
                    ctxnt = colp.tile([128, NCH, 128], F16, tag="ctxnt",
                                      name="ctxnt")
                    nc.vector.tensor_copy(ctxnt[:], pst[:])
                    ctxnts[t] = ctxnt

                def emit_wo(t, cg, store):
                    if t not in store:
                        return
                    ctxnt = store.pop(t)
                    osb = colp.tile([128, E], F32, tag="osb", name="osb")
                    for half in range(2):
                        po = psmm.tile([128, T], F32, tag="mm", name="po")
                        if with_bias:
                            nc.tensor.matmul(
                                po[:, 0:384], ones_sb[:],
                                bo_sb[:, half * 384 : (half + 1) * 384],
                                start=True, stop=False,
                            )
                        for k in range(NCH):
                            nc.tensor.matmul(
                                po[:, 0:384],
                                ctxnt[:, k, :],
                                w_sb["wo"][k][:, half * 384 : (half + 1) * 384],
                                start=(k == 0 and not with_bias),
                                stop=(k == NCH - 1),
                            )
                        nc.scalar.copy(
                            osb[:, half * 384 : (half + 1) * 384], po[:, 0:384]
                        )
                        nc.gpsimd.dma_start(
                            o_d.ap()[cg][:, half * 384 : (half + 1) * 384],
                            osb[:, half * 384 : (half + 1) * 384],
                        )

                emit_scores(0)
                emit_vtrans(0)
                emit_scores(1)
                for t in range(BLK):
                    emit_pv(t)
                    if t + 1 < BLK:
                        emit_vtrans(t + 1)
                    if t + 2 < BLK:
                        emit_scores(t + 2)
                    if t >= 1:
                        emit_wo(t - 1, b * BLK + t - 1, ctxnts)
                    elif pending_wo is not None:
                        pending_wo()           # last column of previous block
                        pending_wo = None
                    emit_norm_tr(t)
                import functools
                pending_wo = functools.partial(
                    emit_wo, BLK - 1, b * BLK + BLK - 1, ctxnts
                )

            if pending_wo is not None:
                pending_wo()
    nc.compile()
    return nc


_PROGRAMS = {}


def _get_program(with_bias: bool, kcap: tuple):
    key = (with_bias, kcap)
    if key not in _PROGRAMS:
        _PROGRAMS[key] = build_program(with_bias, kcap)
    return _PROGRAMS[key]


def plan_columns(mask):
    """mask: (R, C) bool, True = valid key.  Returns per-core column lists,
    per-position capacities, and per-(core,position) key indices."""
    counts = mask.sum(axis=0)                       # (C,)
    order = np.argsort(-counts, kind="stable")
    cols = [[int(order[8 * p + i]) for p in range(CLOC)] for i in range(NCORES)]
    kcap = tuple(
        int(counts[order[8 * p : 8 * p + 8]].max()) for p in range(CLOC)
    )
    return cols, kcap


def make_in_maps(x, self_attn_padding_mask, Wq, bq, Wk, bk, Wv, bv, Wo, bo,
                 with_bias, cols, kcap):
    scaling = float(D) ** -0.5
    def prep_w(W, s=1.0):
        w = (np.asarray(W, np.float32) * s).astype(np.float16)
        return np.ascontiguousarray(w.reshape(NCH, 128, E))
    wq, wk, wv, wo = (prep_w(Wq, scaling), prep_w(Wk), prep_w(Wv), prep_w(Wo))
    mask = np.asarray(self_attn_padding_mask)[0]                   # (R, C)
    xf = np.asarray(x, np.float32)[:, :, 0, :].astype(np.float16)  # (R, C, E)
    ident = np.eye(128, dtype=np.float16)
    off = np.concatenate([[0], np.cumsum(kcap)]).astype(int)
    TOT = int(off[-1])
    in_maps = []
    for i in range(NCORES):
        ci = cols[i]
        # full x^T, position-major tokens
        xs = (
            xf[:, ci]                          # (R, CLOC, E)
            .transpose(1, 0, 2)                # (CLOC, R, E)
            .reshape(NTOK, NCH, 128)
            .transpose(1, 2, 0)                # (NCH, 128, NTOK)
        )
        # packed keys
        xp = np.zeros((TOT, E), np.float16)
        vo = np.zeros((128, CLOC), np.float16)
        for p, col in enumerate(ci):
            idx = np.nonzero(mask[:, col])[0]
            xp[off[p] : off[p] + len(idx)] = xf[idx, col]
            vo[: len(idx), p] = 1.0
        xps = xp.reshape(TOT, NCH, 128).transpose(1, 2, 0)  # (NCH, 128, TOT)
        m = {
            "x": np.ascontiguousarray(xs),
            "xkv": np.ascontiguousarray(xps),
            "vones": vo,
            "wq": wq, "wk": wk, "wv": wv, "wo": wo,
            "ident": ident,
        }
        if with_bias:
            m["bq"] = np.ascontiguousarray(
                (np.asarray(bq, np.float32) * scaling).reshape(NCH, 128).T
            )
            m["bk"] = np.ascontiguousarray(
                np.asarray(bk, np.float32).reshape(NCH, 128).T
            )
            m["bv"] = np.ascontiguousarray(
                np.asarray(bv, np.float32).reshape(NCH, 128).T
            )
            m["bo"] = np.asarray(bo, np.float32).astype(np.float16).reshape(1, E)
        in_maps.append(m)
    return in_maps


def assemble_output(shards, cols):
    out = np.empty((R, C, 1, E), np.float32)
    for i in range(NCORES):
        out[:, cols[i], 0, :] = shards[i].transpose(1, 0, 2)
    return out


def kernel(x, self_attn_padding_mask, Wq, bq, Wk, bk, Wv, bv, Wo, bo):
    global LAST_RESULTS
    with_bias = any(
        bool(np.any(np.asarray(b))) for b in (bq, bk, bv, bo)
    )
    mask = np.asarray(self_attn_padding_mask)[0]
    cols, kcap = plan_columns(mask)
    nc = _get_program(with_bias, kcap)
    in_maps = make_in_maps(
        x, self_attn_padding_mask, Wq, bq, Wk, bk, Wv, bv, Wo, bo, with_bias,
        cols, kcap,
    )
    trace = os.environ.get("KERNEL_TRACE", "") not in ("", "0")
    res = run_bass_kernel_spmd(
        nc, in_maps, core_ids=list(range(NCORES)), trace=trace
    )
    LAST_RESULTS = res
    return assemble_output([res.results[i]["o"] for i in range(NCORES)], cols)
